# revision 27
# baseline (speedup 1.0000x reference)
"""MiniMaxText01 MoE layer on 8 Trainium2 NeuronCores — expert-parallel sparse.

Sharding: core e owns expert e (its w13/w2 slice) and token slice
[512e, 512e+512) for routing + final output.

Per core:
  1. fp32 router on its own 512 tokens (logits -> softmax -> top-2 ->
     renormalize), producing per-token dense weight rows [512, 8].
  2. AllGather the weight rows -> every core has the full [4096, 8] table.
  3. For its expert: mask/cumsum -> slot positions; indirect-DMA scatter
     builds the (token id, weight) slot list; indirect-DMA gather pulls the
     selected token vectors (bf16) from the full x copy in local DRAM.
  4. Sparse SwiGLU MLP on the ~1030 selected tokens (vs 4096 dense):
     PE transposes tokens to feature-major, gate/up matmuls (bf16, fp32
     PSUM), SwiGLU, down-proj, routing-weight scale, transpose back.
  5. Indirect-DMA scatter of weighted outputs into a zeroed [4096, 2048]
     bf16 buffer; ReduceScatter(add) across the 8 cores hands each core
     the summed [512, 2048] slice for its own tokens; upcast to fp32.

Host side only reorders/casts/slices arrays and concatenates the 8
output slices.
"""

import numpy as np
import ml_dtypes

import concourse.bass as bass
import concourse.bass_isa as bass_isa
import concourse.mybir as mybir
import concourse.tile as tile
from concourse.bass_utils import run_bass_kernel_spmd
from concourse.masks import make_identity, make_upper_triangular

# ---------------------------------------------------------------------------
# Workaround: this walrus build rejects instructions carrying >1 sem wait
# ("Too many sync wait commands").  Split the accumulated waits so each
# instruction carries at most one.
from concourse.tile import TileContext
from concourse.vector_clock import ScopedClock


def _drain_and_barrier_split(self, tick_clock, wait_clock):
    drain_inst = self.nc.sync.drain()
    wait_clock.add_sem_waits(
        drain_inst.ins, ScopedClock({None: tick_clock.global_clock})
    )
    inst = drain_inst.ins
    waits = list(inst.sync_info.on_wait)
    if len(waits) > 1:
        inst.sync_info.on_wait = [waits[0]]
        for w in waits[1:]:
            nop = self.nc.sync.nop()
            nop.ins.sync_info = mybir.SyncInfo(on_wait=[w], on_update=[])
    self.nc.all_engine_barrier()
    assert self.sems is not None
    popped = self.nc._tile_sem_poison_stack.pop()
    assert popped is self._sem_poison
    self.nc.clear_and_free_semaphores(list(self.sems.allocated().values()))
    self.nc.all_engine_barrier()


TileContext._drain_and_barrier = _drain_and_barrier_split


def _split_sync_waits(nc, maxw=1):
    import bass_rust

    ctr = 0
    for f in nc.m.functions:
        for bb in f.blocks:
            out = []
            changed = False
            for inst in bb.instructions:
                si = inst.sync_info
                waits = list(si.on_wait) if si is not None else []
                if len(waits) > maxw:
                    for w in waits[:-maxw]:
                        ctr += 1
                        out.append(
                            bass_rust.InstNoOp(
                                name=f"I-wsplit-{ctr}",
                                engine=inst.engine,
                                ins=[],
                                outs=[],
                                sync_info=mybir.SyncInfo(
                                    on_wait=[w], on_update=[]
                                ),
                            )
                        )
                    si.on_wait = waits[-maxw:]
                    changed = True
                out.append(inst)
            if changed:
                bb.instructions = out
# ---------------------------------------------------------------------------

BF16 = ml_dtypes.bfloat16

E, TOPK, H, I = 8, 2, 2048, 5632
T = 4096
NCORES = 8
TC = T // NCORES          # 512 tokens routed per core
P = 128
KH = H // P               # 16  H-chunks
MI = 2 * I // P           # 88  2I m-tiles (g: 0..43, u: 44..87)
NI = I // P               # 44  I-tiles
HT = H // P               # 16  H-tiles
TT = TC // P              # 4   token tiles for the local router
NT = T // P               # 32  token tiles globally
NSEL = 1152               # static per-expert capacity (seen max ~1063)
JT = NSEL // P            # 9   slot tiles
NCH = 3                   # token chunks for the MLP
CH = NSEL // NCH          # 384 tokens per chunk
SENT = 65535.0            # sentinel slot token id: >= T so the output
                          # scatter bounds check skips unused slots

_CACHE = {}


def _build_kernel():
    nc = bass.Bass(num_devices=NCORES)
    f32 = mybir.dt.float32
    bf16 = mybir.dt.bfloat16
    i32 = mybir.dt.int32

    xf_d = nc.dram_tensor("xf", [TT, P, KH * P], f32, kind="ExternalInput")
    gw_d = nc.dram_tensor("gw", [P, KH * E], f32, kind="ExternalInput")
    xg_d = nc.dram_tensor("xg", [T, H], bf16, kind="ExternalInput")
    zb_d = nc.dram_tensor("zb", [1, H], bf16, kind="ExternalInput")
    zf_d = nc.dram_tensor("zf", [1, 2], f32, kind="ExternalInput")
    w13_d = nc.dram_tensor("w13", [MI, P, KH * P], bf16, kind="ExternalInput")
    w2_d = nc.dram_tensor("w2", [HT, P, NI * P], bf16, kind="ExternalInput")
    out_d = nc.dram_tensor("out", [TC, H], f32, kind="ExternalOutput")

    Act = mybir.ActivationFunctionType
    Alu = mybir.AluOpType
    groups = [list(range(NCORES))]

    with tile.TileContext(nc) as tc:
        with (
            tc.tile_pool(name="const", bufs=1) as const_pool,
            tc.tile_pool(name="res", bufs=1) as res_pool,
            tc.tile_pool(name="dram", bufs=1, space="DRAM") as dram_pool,
        ):
            ident_sb = const_pool.tile([P, P], f32)
            make_identity(nc, ident_sb)
            ident_bf = const_pool.tile([P, P], bf16)
            make_identity(nc, ident_bf)
            # tri[k, m] = 1 iff k < m: exclusive per-tile cumsum over
            # partitions via matmul
            tri_sb = const_pool.tile([P, P], f32)
            make_upper_triangular(nc, tri_sb[:], val=1.0, diag=False)
            ones_row = const_pool.tile([1, P], f32)
            nc.vector.memset(ones_row[:], 1.0)
            ones_col = const_pool.tile([P, 1], f32)
            nc.vector.memset(ones_col[:], 1.0)
            gw_sb = const_pool.tile([P, KH, E], f32)
            nc.sync.dma_start(gw_sb[:], gw_d[:].rearrange("p (k e) -> p k e", k=KH))

            # slot data needed across phases
            wtsT_sb = res_pool.tile([E, TT, P], f32)  # local router rows^T
            gidx = res_pool.tile([P, JT], i32)      # slot -> token id (raw)
            gidx_c = res_pool.tile([P, JT], i32)    # clamped for gathers
            wsel_bc = res_pool.tile([P, NSEL], f32)  # slot weight, bcast

            # DRAM scratch
            ybuf = dram_pool.tile([T, H], bf16)          # scatter target
            ybuf_rs = dram_pool.tile([TC, H], bf16)      # ReduceScatter out
            aai = dram_pool.tile([E, TC], f32)           # AllToAll in
            aao = dram_pool.tile([E, TC], f32)           # AllToAll out
            NSC = 4                                      # scatter stripes
            glists = [dram_pool.tile([NSEL, 2], f32, name=f"glist{k}") for k in range(NSC)]

            # zero the scatter stripes early (tiny broadcast DMAs)
            for k in range(NSC):
                nc.scalar.dma_start(
                    glists[k][:], zf_d[0:1, :].to_broadcast([NSEL, 2])
                )
            # zero the output scatter target at the start on the Pool queue:
            # the real 16-engine DMA absorbs these chunks during the router/
            # routing prologue, long before the y scatters need them
            ZCH = 256
            for z in range(T // ZCH):
                nc.gpsimd.dma_start(
                    ybuf[z * ZCH : (z + 1) * ZCH, :],
                    zb_d[0:1, :].to_broadcast([ZCH, H]),
                )

            # ---------------- router (own 512 tokens, fp32) ----------------
            with (
                tc.tile_pool(name="rt", bufs=4) as rt_pool,
                tc.tile_pool(name="rtp", bufs=2, space="PSUM") as rtp_pool,
            ):
                for tt in range(TT):
                    xf_sb = rt_pool.tile([P, KH, P], f32, name="xf")
                    nc.sync.dma_start(
                        xf_sb[:], xf_d[tt].rearrange("p (k t) -> p k t", k=KH)
                    )
                    psum_l = rtp_pool.tile([P, E], f32, name="psl")
                    for k in range(KH):
                        nc.tensor.matmul(
                            psum_l[:],
                            xf_sb[:, k, :],
                            gw_sb[:, k, :],
                            start=(k == 0),
                            stop=(k == KH - 1),
                        )
                    mx = rt_pool.tile([P, 1], f32, name="mx")
                    nc.vector.reduce_max(mx[:], psum_l[:], axis=mybir.AxisListType.X)
                    nmx = rt_pool.tile([P, 1], f32, name="nmx")
                    nc.vector.tensor_scalar_mul(nmx[:], mx[:], -1.0)
                    p_sb = rt_pool.tile([P, E], f32, name="p")
                    nc.scalar.activation(p_sb[:], psum_l[:], Act.Exp, bias=nmx[:])
                    m1 = rt_pool.tile([P, 1], f32, name="m1")
                    nc.vector.reduce_max(m1[:], p_sb[:], axis=mybir.AxisListType.X)
                    pm = rt_pool.tile([P, E], f32, name="pm")
                    nc.vector.scalar_tensor_tensor(
                        pm[:], p_sb[:], m1[:], p_sb[:], Alu.is_lt, Alu.mult
                    )
                    m2 = rt_pool.tile([P, 1], f32, name="m2")
                    nc.vector.reduce_max(m2[:], pm[:], axis=mybir.AxisListType.X)
                    denom = rt_pool.tile([P, 1], f32, name="den")
                    nc.vector.tensor_add(denom[:], m1[:], m2[:])
                    rden = rt_pool.tile([P, 1], f32, name="rden")
                    nc.vector.reciprocal(rden[:], denom[:])
                    wts = rt_pool.tile([P, E], f32, name="wts")
                    nc.vector.scalar_tensor_tensor(
                        wts[:], p_sb[:], m2[:], p_sb[:], Alu.is_ge, Alu.mult
                    )
                    nc.vector.tensor_scalar_mul(wts[:], wts[:], rden[:])
                    ps_w = rtp_pool.tile([E, P], f32, name="psw")
                    nc.tensor.transpose(ps_w[:], wts[:], ident_sb[:])
                    nc.vector.tensor_copy(wtsT_sb[:, tt, :], ps_w[:])
                nc.gpsimd.dma_start(
                    aai[:], wtsT_sb[:].rearrange("e t p -> e (t p)")
                )

            # ---------------- AllToAll: my expert's weights, all tokens ---
            nc.gpsimd.collective_compute(
                "AllToAll",
                Alu.bypass,
                replica_groups=groups,
                ins=[aai[:].opt()],
                outs=[aao[:].opt()],
            )

            # ---------------- routing data for my expert ----------------
            with (
                tc.tile_pool(name="xep", bufs=1) as xe_pool,
            ):
                x_e = xe_pool.tile([P, KH, NSEL], bf16)

                with (
                    tc.tile_pool(name="route", bufs=1) as route_pool,
                    tc.tile_pool(name="rp", bufs=1, space="PSUM") as rp_pool,
                    tc.tile_pool(name="tp", bufs=2, space="PSUM") as tp_pool,
                ):
                    we_all = route_pool.tile([P, NT], f32)  # my expert weight
                    mask = route_pool.tile([P, NT], f32)    # selected?
                    nc.sync.dma_start(
                        we_all[:].rearrange("p (s u) -> p s u", s=E),
                        aao[:].rearrange("s (u p) -> p s u", p=P),
                    )
                    nc.vector.tensor_scalar(
                        mask[:], we_all[:], 0.0, None, op0=Alu.is_gt
                    )
                    # exclusive cumsum over partitions within each tile
                    # (matmul with strict triangular)
                    pos_psum = rp_pool.tile([P, NT], f32)
                    for tt in range(NT):
                        nc.tensor.matmul(
                            pos_psum[:, tt : tt + 1],
                            tri_sb[:],
                            mask[:, tt : tt + 1],
                            start=True,
                            stop=True,
                        )
                    pos_excl = route_pool.tile([P, NT], f32)
                    nc.vector.tensor_copy(pos_excl[:], pos_psum[:])
                    # per-tile totals via a column-sum matmul
                    cnt_psum = rp_pool.tile([1, NT], f32)
                    nc.tensor.matmul(
                        cnt_psum[:], ones_col[:], mask[:], start=True, stop=True
                    )
                    cnt_row = route_pool.tile([1, NT], f32)
                    nc.vector.tensor_copy(cnt_row[:], cnt_psum[:])
                    # exclusive running carry across tiles (serial tiny adds)
                    carry_row = route_pool.tile([1, NT], f32)
                    nc.vector.memset(carry_row[0:1, 0:1], 0.0)
                    for tt in range(1, NT):
                        nc.vector.tensor_add(
                            carry_row[0:1, tt : tt + 1],
                            carry_row[0:1, tt - 1 : tt],
                            cnt_row[0:1, tt - 1 : tt],
                        )
                    # broadcast the carry across partitions
                    carry_psum = rp_pool.tile([P, NT], f32)
                    nc.tensor.matmul(
                        carry_psum[:], ones_row[:], carry_row[:],
                        start=True, stop=True,
                    )
                    # non-selected tokens -> position SENT (scatter skips)
                    notmask = route_pool.tile([P, NT], f32)
                    nc.vector.tensor_scalar(
                        notmask[:], mask[:], 0.0, None, op0=Alu.is_equal
                    )
                    nc.vector.tensor_scalar_mul(notmask[:], notmask[:], SENT)
                    pos_f = route_pool.tile([P, NT], f32)
                    nc.vector.tensor_add(pos_f[:], pos_excl[:], carry_psum[:])
                    nc.vector.tensor_add(pos_f[:], pos_f[:], notmask[:])
                    pos_i = route_pool.tile([P, NT], i32)
                    nc.vector.tensor_copy(pos_i[:], pos_f[:])

                    # scatter (token id, weight) into glist at slot positions
                    tok_i = route_pool.tile([P, NT], i32)
                    nc.gpsimd.iota(
                        tok_i[:], pattern=[[P, NT]], base=0, channel_multiplier=1
                    )
                    tok_f = route_pool.tile([P, NT], f32)
                    nc.vector.tensor_copy(tok_f[:], tok_i[:])
                    payload = route_pool.tile([P, NT, 2], f32)
                    nc.vector.tensor_copy(payload[:, :, 0], tok_f[:])
                    nc.vector.tensor_copy(payload[:, :, 1], we_all[:])
                    for tt in range(NT):
                        nc.gpsimd.indirect_dma_start(
                            out=glists[tt % NSC][:],
                            out_offset=bass.IndirectOffsetOnAxis(
                                ap=pos_i[:, tt : tt + 1], axis=0
                            ),
                            in_=payload[:, tt, :],
                            in_offset=None,
                            bounds_check=NSEL - 1,
                            oob_is_err=False,
                        )

                    # read the stripes back and merge (slots are disjoint;
                    # unused entries are zero in every stripe)
                    gl4 = []
                    for k in range(NSC):
                        g = route_pool.tile([P, JT, 2], f32, name=f"gl{k}")
                        nc.sync.dma_start(
                            g[:], glists[k][:].rearrange("(j p) c -> p j c", p=P)
                        )
                        gl4.append(g)
                    m01 = route_pool.tile([P, JT, 2], f32)
                    m23 = route_pool.tile([P, JT, 2], f32)
                    glm = route_pool.tile([P, JT, 2], f32)
                    nc.vector.tensor_add(
                        m01[:].rearrange("p j c -> p (j c)"),
                        gl4[0][:].rearrange("p j c -> p (j c)"),
                        gl4[1][:].rearrange("p j c -> p (j c)"),
                    )
                    nc.vector.tensor_add(
                        m23[:].rearrange("p j c -> p (j c)"),
                        gl4[2][:].rearrange("p j c -> p (j c)"),
                        gl4[3][:].rearrange("p j c -> p (j c)"),
                    )
                    nc.vector.tensor_add(
                        glm[:].rearrange("p j c -> p (j c)"),
                        m01[:].rearrange("p j c -> p (j c)"),
                        m23[:].rearrange("p j c -> p (j c)"),
                    )
                    wsel = route_pool.tile([P, JT], f32)
                    nc.vector.tensor_copy(wsel[:], glm[:, :, 1])
                    # unused slots have w == 0 -> token id SENT (skipped by
                    # the output scatter's bounds check)
                    wz = route_pool.tile([P, JT], f32)
                    nc.vector.tensor_scalar(
                        wz[:], wsel[:], 0.0, None, op0=Alu.is_equal
                    )
                    nc.vector.tensor_scalar_mul(wz[:], wz[:], SENT)
                    tfix = route_pool.tile([P, JT], f32)
                    nc.vector.tensor_add(tfix[:], glm[:, :, 0], wz[:])
                    nc.vector.tensor_copy(gidx[:], tfix[:])
                    nc.vector.tensor_scalar_min(gidx_c[:], gidx[:], T - 1)

                    # broadcast slot weights: wsel_bc[:, j] = weight(slot j)
                    wselT = route_pool.tile([1, JT, P], f32)
                    for j in range(JT):
                        ps_t = tp_pool.tile([1, P], f32, name="pst")
                        nc.tensor.transpose(
                            ps_t[:], wsel[:, j : j + 1], ident_sb[:]
                        )
                        nc.vector.tensor_copy(wselT[0:1, j, :], ps_t[:])
                    wselT_flat = wselT[:].rearrange("o j p -> o (j p)")
                    for c in range(NCH):
                        ps_b = tp_pool.tile([P, CH], f32, name="psb")
                        nc.tensor.matmul(
                            ps_b[:],
                            ones_row[:],
                            wselT_flat[:, c * CH : (c + 1) * CH],
                            start=True,
                            stop=True,
                        )
                        nc.scalar.copy(wsel_bc[:, c * CH : (c + 1) * CH], ps_b[:])

                # gather selected tokens (bf16 rows of the full x) and
                # transpose feature-major
                with (
                    tc.tile_pool(name="xgp", bufs=3) as xg_pool,
                    tc.tile_pool(name="xtp", bufs=2, space="PSUM") as xt_psum,
                ):
                    for j in range(JT):
                        xg_sb = xg_pool.tile([P, H], bf16, name="xg")
                        nc.gpsimd.indirect_dma_start(
                            out=xg_sb[:],
                            out_offset=None,
                            in_=xg_d[0:P, :],
                            in_offset=bass.IndirectOffsetOnAxis(
                                ap=gidx_c[:, j : j + 1], axis=0
                            ),
                        )
                        for k in range(KH):
                            ps_x = xt_psum.tile([P, P], bf16, name="psx")
                            nc.tensor.transpose(
                                ps_x[:],
                                xg_sb[:, k * P : (k + 1) * P],
                                ident_bf[:],
                            )
                            nc.vector.tensor_copy(
                                x_e[:, k, j * P : (j + 1) * P], ps_x[:]
                            )

                # ---------------- gate/up + SwiGLU ----------------
                h_sb = res_pool.tile([P, NI, NSEL], bf16)
                with (
                    tc.tile_pool(name="w13p", bufs=4) as w13_pool,
                    tc.tile_pool(name="gup", bufs=4, space="PSUM") as gu_psum,
                    tc.tile_pool(name="sgp", bufs=4) as sg_pool,
                ):
                    for i in range(NI):
                        wg = w13_pool.tile([P, KH, P], bf16, name="w13")
                        nc.sync.dma_start(
                            wg[:], w13_d[i].rearrange("p (k m) -> p k m", k=KH)
                        )
                        wu = w13_pool.tile([P, KH, P], bf16, name="w13")
                        nc.sync.dma_start(
                            wu[:],
                            w13_d[NI + i].rearrange("p (k m) -> p k m", k=KH),
                        )
                        for c in range(NCH):
                            sl = slice(c * CH, (c + 1) * CH)
                            psum_g = gu_psum.tile([P, CH], f32, name="gu")
                            psum_u = gu_psum.tile([P, CH], f32, name="gu")
                            for k in range(KH):
                                nc.tensor.matmul(
                                    psum_g[:],
                                    wg[:, k, :],
                                    x_e[:, k, sl],
                                    start=(k == 0),
                                    stop=(k == KH - 1),
                                )
                            for k in range(KH):
                                nc.tensor.matmul(
                                    psum_u[:],
                                    wu[:, k, :],
                                    x_e[:, k, sl],
                                    start=(k == 0),
                                    stop=(k == KH - 1),
                                )
                            sg = sg_pool.tile([P, CH], f32, name="sg")
                            nc.scalar.activation(sg[:], psum_g[:], Act.Silu)
                            nc.vector.tensor_tensor(
                                h_sb[:, i, sl], sg[:], psum_u[:], Alu.mult
                            )

            # ---------------- down-proj, weight, transpose, scatter -------
            with (
                tc.tile_pool(name="ydp", bufs=1) as y_pool,
                tc.tile_pool(name="w2p", bufs=3) as w2_pool,
                tc.tile_pool(name="yp", bufs=4, space="PSUM") as y_psum,
                tc.tile_pool(name="ytp", bufs=3) as yt_pool,
                tc.tile_pool(name="ytps", bufs=2, space="PSUM") as yt_psum,
            ):
                y_sb = y_pool.tile([P, HT, NSEL], bf16)
                for hh in range(HT):
                    w2t = w2_pool.tile([P, NI, P], bf16, name="w2")
                    nc.sync.dma_start(
                        w2t[:], w2_d[hh].rearrange("p (i m) -> p i m", i=NI)
                    )
                    for c in range(NCH):
                        sl = slice(c * CH, (c + 1) * CH)
                        psum_y = y_psum.tile([P, CH], f32, name="py")
                        for i in range(NI):
                            nc.tensor.matmul(
                                psum_y[:],
                                w2t[:, i, :],
                                h_sb[:, i, sl],
                                start=(i == 0),
                                stop=(i == NI - 1),
                            )
                        nc.vector.tensor_tensor(
                            y_sb[:, hh, sl], psum_y[:], wsel_bc[:, sl], Alu.mult
                        )
                # transpose token-major and scatter into ybuf
                for j in range(JT):
                    yt = yt_pool.tile([P, H], bf16, name="yt")
                    for k in range(HT):
                        ps_y = yt_psum.tile([P, P], bf16, name="psy")
                        nc.tensor.transpose(
                            ps_y[:],
                            y_sb[:, k, j * P : (j + 1) * P],
                            ident_bf[:],
                        )
                        nc.scalar.copy(yt[:, k * P : (k + 1) * P], ps_y[:])
                    nc.gpsimd.indirect_dma_start(
                        out=ybuf[0:P, :],
                        out_offset=bass.IndirectOffsetOnAxis(
                            ap=gidx[:, j : j + 1], axis=0
                        ),
                        in_=yt[:],
                        in_offset=None,
                        bounds_check=T - 1,
                        oob_is_err=False,
                    )

            # ---------------- combine ----------------
            nc.gpsimd.collective_compute(
                "ReduceScatter",
                Alu.add,
                replica_groups=groups,
                ins=[ybuf[:].opt()],
                outs=[ybuf_rs[:].opt()],
            )
            with tc.tile_pool(name="up", bufs=2) as up_pool:
                for tt in range(TT):
                    yb = up_pool.tile([P, H], bf16, name="yb")
                    nc.sync.dma_start(yb[:], ybuf_rs[tt * P : (tt + 1) * P, :])
                    yf = up_pool.tile([P, H], f32, name="yf")
                    nc.vector.tensor_copy(yf[:], yb[:])
                    nc.sync.dma_start(out_d[tt * P : (tt + 1) * P, :], yf[:])

    return nc


def _prep_inputs(hidden_states, gate_w, w13, w2):
    """Host-side shard/layout prep (cast + transpose/slice only, no math)."""
    x = np.asarray(hidden_states, np.float32)
    gate_w = np.asarray(gate_w, np.float32)

    gw = np.ascontiguousarray(
        gate_w.reshape(E, KH, P).transpose(2, 1, 0).reshape(P, KH * E)
    )
    xg = np.ascontiguousarray(x.astype(BF16))

    w13b = np.asarray(w13).astype(BF16)
    w2b = np.asarray(w2).astype(BF16)

    in_maps = []
    for c in range(NCORES):
        xc = x[c * TC : (c + 1) * TC]
        xf = np.ascontiguousarray(
            xc.reshape(TT, P, KH, P).transpose(0, 3, 2, 1).reshape(TT, P, KH * P)
        )
        w13d = np.ascontiguousarray(
            w13b[c].reshape(MI, P, KH, P).transpose(0, 3, 2, 1).reshape(
                MI, P, KH * P
            )
        )
        w2d = np.ascontiguousarray(
            w2b[c].reshape(HT, P, NI, P).transpose(0, 3, 2, 1).reshape(
                HT, P, NI * P
            )
        )
        in_maps.append(
            {
                "xf": xf,
                "gw": gw,
                "xg": xg,
                "zb": np.zeros((1, H), BF16),
                "zf": np.zeros((1, 2), np.float32),
                "w13": w13d,
                "w2": w2d,
            }
        )
    return in_maps


def kernel(hidden_states, gate_w, w13, w2, top_k):
    import time

    assert int(top_k) == TOPK
    t0 = time.time()
    if "nc" not in _CACHE:
        nc = _build_kernel()
        _split_sync_waits(nc)
        _CACHE["nc"] = nc
    nc = _CACHE["nc"]
    t1 = time.time()
    in_maps = _prep_inputs(hidden_states, gate_w, w13, w2)
    t2 = time.time()
    res = run_bass_kernel_spmd(nc, in_maps, core_ids=list(range(NCORES)))
    t3 = time.time()
    print(
        f"[kernel] build {t1 - t0:.1f}s  prep {t2 - t1:.1f}s  run {t3 - t2:.1f}s",
        flush=True,
    )
    _CACHE["last_results"] = res

    out = np.empty((T, H), np.float32)
    for c in range(NCORES):
        out[c * TC : (c + 1) * TC] = res.results[c]["out"]
    return out


# revision 28
# speedup vs baseline: 1.0390x; 1.0390x over previous
"""MiniMaxText01 MoE layer on 8 Trainium2 NeuronCores — expert-parallel sparse.

Sharding: core e owns expert e (its w13/w2 slice) and token slice
[512e, 512e+512) for routing + final output.

Per core:
  1. fp32 router on its own 512 tokens (logits -> softmax -> top-2 ->
     renormalize), producing per-token dense weight rows [512, 8].
  2. AllGather the weight rows -> every core has the full [4096, 8] table.
  3. For its expert: mask/cumsum -> slot positions; indirect-DMA scatter
     builds the (token id, weight) slot list; indirect-DMA gather pulls the
     selected token vectors (bf16) from the full x copy in local DRAM.
  4. Sparse SwiGLU MLP on the ~1030 selected tokens (vs 4096 dense):
     PE transposes tokens to feature-major, gate/up matmuls (bf16, fp32
     PSUM), SwiGLU, down-proj, routing-weight scale, transpose back.
  5. Indirect-DMA scatter of weighted outputs into a zeroed [4096, 2048]
     bf16 buffer; ReduceScatter(add) across the 8 cores hands each core
     the summed [512, 2048] slice for its own tokens; upcast to fp32.

Host side only reorders/casts/slices arrays and concatenates the 8
output slices.
"""

import numpy as np
import ml_dtypes

import concourse.bass as bass
import concourse.bass_isa as bass_isa
import concourse.mybir as mybir
import concourse.tile as tile
from concourse.bass_utils import run_bass_kernel_spmd
from concourse.masks import make_identity, make_upper_triangular

# ---------------------------------------------------------------------------
# Workaround: this walrus build rejects instructions carrying >1 sem wait
# ("Too many sync wait commands").  Split the accumulated waits so each
# instruction carries at most one.
from concourse.tile import TileContext
from concourse.vector_clock import ScopedClock


def _drain_and_barrier_split(self, tick_clock, wait_clock):
    drain_inst = self.nc.sync.drain()
    wait_clock.add_sem_waits(
        drain_inst.ins, ScopedClock({None: tick_clock.global_clock})
    )
    inst = drain_inst.ins
    waits = list(inst.sync_info.on_wait)
    if len(waits) > 1:
        inst.sync_info.on_wait = [waits[0]]
        for w in waits[1:]:
            nop = self.nc.sync.nop()
            nop.ins.sync_info = mybir.SyncInfo(on_wait=[w], on_update=[])
    self.nc.all_engine_barrier()
    assert self.sems is not None
    popped = self.nc._tile_sem_poison_stack.pop()
    assert popped is self._sem_poison
    self.nc.clear_and_free_semaphores(list(self.sems.allocated().values()))
    self.nc.all_engine_barrier()


TileContext._drain_and_barrier = _drain_and_barrier_split


def _split_sync_waits(nc, maxw=1):
    import bass_rust

    ctr = 0
    for f in nc.m.functions:
        for bb in f.blocks:
            out = []
            changed = False
            for inst in bb.instructions:
                si = inst.sync_info
                waits = list(si.on_wait) if si is not None else []
                if len(waits) > maxw:
                    for w in waits[:-maxw]:
                        ctr += 1
                        out.append(
                            bass_rust.InstNoOp(
                                name=f"I-wsplit-{ctr}",
                                engine=inst.engine,
                                ins=[],
                                outs=[],
                                sync_info=mybir.SyncInfo(
                                    on_wait=[w], on_update=[]
                                ),
                            )
                        )
                    si.on_wait = waits[-maxw:]
                    changed = True
                out.append(inst)
            if changed:
                bb.instructions = out
# ---------------------------------------------------------------------------

BF16 = ml_dtypes.bfloat16

E, TOPK, H, I = 8, 2, 2048, 5632
T = 4096
NCORES = 8
TC = T // NCORES          # 512 tokens routed per core
P = 128
KH = H // P               # 16  H-chunks
MI = 2 * I // P           # 88  2I m-tiles (g: 0..43, u: 44..87)
NI = I // P               # 44  I-tiles
HT = H // P               # 16  H-tiles
TT = TC // P              # 4   token tiles for the local router
NT = T // P               # 32  token tiles globally
NSEL = 1152               # static per-expert capacity (seen max ~1063)
JT = NSEL // P            # 9   slot tiles
NCH = 3                   # token chunks for the MLP
CH = NSEL // NCH          # 384 tokens per chunk
SENT = 65535.0            # sentinel slot token id: >= T so the output
                          # scatter bounds check skips unused slots

_CACHE = {}


def _build_kernel():
    nc = bass.Bass(num_devices=NCORES)
    f32 = mybir.dt.float32
    bf16 = mybir.dt.bfloat16
    i32 = mybir.dt.int32

    xf_d = nc.dram_tensor("xf", [TT, P, KH * P], f32, kind="ExternalInput")
    gw_d = nc.dram_tensor("gw", [P, KH * E], f32, kind="ExternalInput")
    xg_d = nc.dram_tensor("xg", [T, H], bf16, kind="ExternalInput")
    zb_d = nc.dram_tensor("zb", [1, H], bf16, kind="ExternalInput")
    zf_d = nc.dram_tensor("zf", [1, 2], f32, kind="ExternalInput")
    w13_d = nc.dram_tensor("w13", [MI, P, KH * P], bf16, kind="ExternalInput")
    w2_d = nc.dram_tensor("w2", [HT, P, NI * P], bf16, kind="ExternalInput")
    out_d = nc.dram_tensor("out", [TC, H], f32, kind="ExternalOutput")

    Act = mybir.ActivationFunctionType
    Alu = mybir.AluOpType
    groups = [list(range(NCORES))]

    with tile.TileContext(nc) as tc:
        with (
            tc.tile_pool(name="const", bufs=1) as const_pool,
            tc.tile_pool(name="res", bufs=1) as res_pool,
            tc.tile_pool(name="dram", bufs=1, space="DRAM") as dram_pool,
        ):
            ident_sb = const_pool.tile([P, P], f32)
            make_identity(nc, ident_sb)
            ident_bf = const_pool.tile([P, P], bf16)
            make_identity(nc, ident_bf)
            # tri[k, m] = 1 iff k < m: exclusive per-tile cumsum over
            # partitions via matmul
            tri_sb = const_pool.tile([P, P], f32)
            make_upper_triangular(nc, tri_sb[:], val=1.0, diag=False)
            ones_row = const_pool.tile([1, P], f32)
            nc.vector.memset(ones_row[:], 1.0)
            ones_col = const_pool.tile([P, 1], f32)
            nc.vector.memset(ones_col[:], 1.0)
            gw_sb = const_pool.tile([P, KH, E], f32)
            nc.sync.dma_start(gw_sb[:], gw_d[:].rearrange("p (k e) -> p k e", k=KH))

            # slot data needed across phases
            wtsT_sb = res_pool.tile([E, TT, P], f32)  # local router rows^T
            gidx = res_pool.tile([P, JT], i32)      # slot -> token id (raw)
            gidx_c = res_pool.tile([P, JT], i32)    # clamped for gathers
            wsel_bc = res_pool.tile([P, NSEL], f32)  # slot weight, bcast

            # DRAM scratch
            ybuf = dram_pool.tile([T, H], bf16)          # scatter target
            ybuf_rs = dram_pool.tile([TC, H], bf16)      # ReduceScatter out
            aai = dram_pool.tile([E, TC], f32)           # AllToAll in
            aao = dram_pool.tile([E, TC], f32)           # AllToAll out
            NSC = 4                                      # scatter stripes
            glists = [dram_pool.tile([NSEL, 2], f32, name=f"glist{k}") for k in range(NSC)]

            # zero the scatter stripes early (tiny broadcast DMAs)
            for k in range(NSC):
                nc.scalar.dma_start(
                    glists[k][:], zf_d[0:1, :].to_broadcast([NSEL, 2])
                )

            # ---------------- router (own 512 tokens, fp32) ----------------
            with (
                tc.tile_pool(name="rt", bufs=4) as rt_pool,
                tc.tile_pool(name="rtp", bufs=2, space="PSUM") as rtp_pool,
            ):
                for tt in range(TT):
                    xf_sb = rt_pool.tile([P, KH, P], f32, name="xf")
                    nc.sync.dma_start(
                        xf_sb[:], xf_d[tt].rearrange("p (k t) -> p k t", k=KH)
                    )
                    psum_l = rtp_pool.tile([P, E], f32, name="psl")
                    for k in range(KH):
                        nc.tensor.matmul(
                            psum_l[:],
                            xf_sb[:, k, :],
                            gw_sb[:, k, :],
                            start=(k == 0),
                            stop=(k == KH - 1),
                        )
                    mx = rt_pool.tile([P, 1], f32, name="mx")
                    nc.vector.reduce_max(mx[:], psum_l[:], axis=mybir.AxisListType.X)
                    nmx = rt_pool.tile([P, 1], f32, name="nmx")
                    nc.vector.tensor_scalar_mul(nmx[:], mx[:], -1.0)
                    p_sb = rt_pool.tile([P, E], f32, name="p")
                    nc.scalar.activation(p_sb[:], psum_l[:], Act.Exp, bias=nmx[:])
                    m1 = rt_pool.tile([P, 1], f32, name="m1")
                    nc.vector.reduce_max(m1[:], p_sb[:], axis=mybir.AxisListType.X)
                    pm = rt_pool.tile([P, E], f32, name="pm")
                    nc.vector.scalar_tensor_tensor(
                        pm[:], p_sb[:], m1[:], p_sb[:], Alu.is_lt, Alu.mult
                    )
                    m2 = rt_pool.tile([P, 1], f32, name="m2")
                    nc.vector.reduce_max(m2[:], pm[:], axis=mybir.AxisListType.X)
                    denom = rt_pool.tile([P, 1], f32, name="den")
                    nc.vector.tensor_add(denom[:], m1[:], m2[:])
                    rden = rt_pool.tile([P, 1], f32, name="rden")
                    nc.vector.reciprocal(rden[:], denom[:])
                    wts = rt_pool.tile([P, E], f32, name="wts")
                    nc.vector.scalar_tensor_tensor(
                        wts[:], p_sb[:], m2[:], p_sb[:], Alu.is_ge, Alu.mult
                    )
                    nc.vector.tensor_scalar_mul(wts[:], wts[:], rden[:])
                    ps_w = rtp_pool.tile([E, P], f32, name="psw")
                    nc.tensor.transpose(ps_w[:], wts[:], ident_sb[:])
                    nc.vector.tensor_copy(wtsT_sb[:, tt, :], ps_w[:])
                nc.gpsimd.dma_start(
                    aai[:], wtsT_sb[:].rearrange("e t p -> e (t p)")
                )

            # ---------------- AllToAll: my expert's weights, all tokens ---
            nc.gpsimd.collective_compute(
                "AllToAll",
                Alu.bypass,
                replica_groups=groups,
                ins=[aai[:].opt()],
                outs=[aao[:].opt()],
            )

            # ---------------- routing data for my expert ----------------
            with (
                tc.tile_pool(name="xep", bufs=1) as xe_pool,
            ):
                x_e = xe_pool.tile([P, KH, NSEL], bf16)

                with (
                    tc.tile_pool(name="route", bufs=1) as route_pool,
                    tc.tile_pool(name="rp", bufs=1, space="PSUM") as rp_pool,
                    tc.tile_pool(name="tp", bufs=2, space="PSUM") as tp_pool,
                ):
                    we_all = route_pool.tile([P, NT], f32)  # my expert weight
                    mask = route_pool.tile([P, NT], f32)    # selected?
                    nc.sync.dma_start(
                        we_all[:].rearrange("p (s u) -> p s u", s=E),
                        aao[:].rearrange("s (u p) -> p s u", p=P),
                    )
                    nc.vector.tensor_scalar(
                        mask[:], we_all[:], 0.0, None, op0=Alu.is_gt
                    )
                    # exclusive cumsum over partitions within each tile
                    # (matmul with strict triangular)
                    pos_psum = rp_pool.tile([P, NT], f32)
                    for tt in range(NT):
                        nc.tensor.matmul(
                            pos_psum[:, tt : tt + 1],
                            tri_sb[:],
                            mask[:, tt : tt + 1],
                            start=True,
                            stop=True,
                        )
                    pos_excl = route_pool.tile([P, NT], f32)
                    nc.vector.tensor_copy(pos_excl[:], pos_psum[:])
                    # per-tile totals via a column-sum matmul
                    cnt_psum = rp_pool.tile([1, NT], f32)
                    nc.tensor.matmul(
                        cnt_psum[:], ones_col[:], mask[:], start=True, stop=True
                    )
                    cnt_row = route_pool.tile([1, NT], f32)
                    nc.vector.tensor_copy(cnt_row[:], cnt_psum[:])
                    # exclusive running carry across tiles (serial tiny adds)
                    carry_row = route_pool.tile([1, NT], f32)
                    nc.vector.memset(carry_row[0:1, 0:1], 0.0)
                    for tt in range(1, NT):
                        nc.vector.tensor_add(
                            carry_row[0:1, tt : tt + 1],
                            carry_row[0:1, tt - 1 : tt],
                            cnt_row[0:1, tt - 1 : tt],
                        )
                    # broadcast the carry across partitions
                    carry_psum = rp_pool.tile([P, NT], f32)
                    nc.tensor.matmul(
                        carry_psum[:], ones_row[:], carry_row[:],
                        start=True, stop=True,
                    )
                    # non-selected tokens -> position SENT (scatter skips)
                    notmask = route_pool.tile([P, NT], f32)
                    nc.vector.tensor_scalar(
                        notmask[:], mask[:], 0.0, None, op0=Alu.is_equal
                    )
                    nc.vector.tensor_scalar_mul(notmask[:], notmask[:], SENT)
                    pos_f = route_pool.tile([P, NT], f32)
                    nc.vector.tensor_add(pos_f[:], pos_excl[:], carry_psum[:])
                    nc.vector.tensor_add(pos_f[:], pos_f[:], notmask[:])
                    pos_i = route_pool.tile([P, NT], i32)
                    nc.vector.tensor_copy(pos_i[:], pos_f[:])

                    # scatter (token id, weight) into glist at slot positions
                    tok_i = route_pool.tile([P, NT], i32)
                    nc.gpsimd.iota(
                        tok_i[:], pattern=[[P, NT]], base=0, channel_multiplier=1
                    )
                    tok_f = route_pool.tile([P, NT], f32)
                    nc.vector.tensor_copy(tok_f[:], tok_i[:])
                    payload = route_pool.tile([P, NT, 2], f32)
                    nc.vector.tensor_copy(payload[:, :, 0], tok_f[:])
                    nc.vector.tensor_copy(payload[:, :, 1], we_all[:])
                    for tt in range(NT):
                        nc.gpsimd.indirect_dma_start(
                            out=glists[tt % NSC][:],
                            out_offset=bass.IndirectOffsetOnAxis(
                                ap=pos_i[:, tt : tt + 1], axis=0
                            ),
                            in_=payload[:, tt, :],
                            in_offset=None,
                            bounds_check=NSEL - 1,
                            oob_is_err=False,
                        )

                    # read the stripes back and merge (slots are disjoint;
                    # unused entries are zero in every stripe)
                    gl4 = []
                    for k in range(NSC):
                        g = route_pool.tile([P, JT, 2], f32, name=f"gl{k}")
                        nc.sync.dma_start(
                            g[:], glists[k][:].rearrange("(j p) c -> p j c", p=P)
                        )
                        gl4.append(g)
                    m01 = route_pool.tile([P, JT, 2], f32)
                    m23 = route_pool.tile([P, JT, 2], f32)
                    glm = route_pool.tile([P, JT, 2], f32)
                    nc.vector.tensor_add(
                        m01[:].rearrange("p j c -> p (j c)"),
                        gl4[0][:].rearrange("p j c -> p (j c)"),
                        gl4[1][:].rearrange("p j c -> p (j c)"),
                    )
                    nc.vector.tensor_add(
                        m23[:].rearrange("p j c -> p (j c)"),
                        gl4[2][:].rearrange("p j c -> p (j c)"),
                        gl4[3][:].rearrange("p j c -> p (j c)"),
                    )
                    nc.vector.tensor_add(
                        glm[:].rearrange("p j c -> p (j c)"),
                        m01[:].rearrange("p j c -> p (j c)"),
                        m23[:].rearrange("p j c -> p (j c)"),
                    )
                    wsel = route_pool.tile([P, JT], f32)
                    nc.vector.tensor_copy(wsel[:], glm[:, :, 1])
                    # unused slots have w == 0 -> token id SENT (skipped by
                    # the output scatter's bounds check)
                    wz = route_pool.tile([P, JT], f32)
                    nc.vector.tensor_scalar(
                        wz[:], wsel[:], 0.0, None, op0=Alu.is_equal
                    )
                    nc.vector.tensor_scalar_mul(wz[:], wz[:], SENT)
                    tfix = route_pool.tile([P, JT], f32)
                    nc.vector.tensor_add(tfix[:], glm[:, :, 0], wz[:])
                    nc.vector.tensor_copy(gidx[:], tfix[:])
                    nc.vector.tensor_scalar_min(gidx_c[:], gidx[:], T - 1)

                    # broadcast slot weights: wsel_bc[:, j] = weight(slot j)
                    wselT = route_pool.tile([1, JT, P], f32)
                    for j in range(JT):
                        ps_t = tp_pool.tile([1, P], f32, name="pst")
                        nc.tensor.transpose(
                            ps_t[:], wsel[:, j : j + 1], ident_sb[:]
                        )
                        nc.vector.tensor_copy(wselT[0:1, j, :], ps_t[:])
                    wselT_flat = wselT[:].rearrange("o j p -> o (j p)")
                    for c in range(NCH):
                        ps_b = tp_pool.tile([P, CH], f32, name="psb")
                        nc.tensor.matmul(
                            ps_b[:],
                            ones_row[:],
                            wselT_flat[:, c * CH : (c + 1) * CH],
                            start=True,
                            stop=True,
                        )
                        nc.scalar.copy(wsel_bc[:, c * CH : (c + 1) * CH], ps_b[:])

                # zero the output scatter target; only needed before the y
                # scatters ~1ms from now.  Chunked so other DMA traffic can
                # interleave.
                ZCH = 256
                for z in range(T // ZCH):
                    nc.scalar.dma_start(
                        ybuf[z * ZCH : (z + 1) * ZCH, :],
                        zb_d[0:1, :].to_broadcast([ZCH, H]),
                    )

                # gather selected tokens (bf16 rows of the full x) and
                # transpose feature-major
                with (
                    tc.tile_pool(name="xgp", bufs=3) as xg_pool,
                    tc.tile_pool(name="xtp", bufs=2, space="PSUM") as xt_psum,
                ):
                    for j in range(JT):
                        xg_sb = xg_pool.tile([P, H], bf16, name="xg")
                        nc.gpsimd.indirect_dma_start(
                            out=xg_sb[:],
                            out_offset=None,
                            in_=xg_d[0:P, :],
                            in_offset=bass.IndirectOffsetOnAxis(
                                ap=gidx_c[:, j : j + 1], axis=0
                            ),
                        )
                        for k in range(KH):
                            ps_x = xt_psum.tile([P, P], bf16, name="psx")
                            nc.tensor.transpose(
                                ps_x[:],
                                xg_sb[:, k * P : (k + 1) * P],
                                ident_bf[:],
                            )
                            nc.scalar.copy(
                                x_e[:, k, j * P : (j + 1) * P], ps_x[:]
                            )

                # ---------------- gate/up + SwiGLU ----------------
                h_sb = res_pool.tile([P, NI, NSEL], bf16)
                with (
                    tc.tile_pool(name="w13p", bufs=4) as w13_pool,
                    tc.tile_pool(name="gup", bufs=4, space="PSUM") as gu_psum,
                    tc.tile_pool(name="sgp", bufs=4) as sg_pool,
                ):
                    for i in range(NI):
                        wg = w13_pool.tile([P, KH, P], bf16, name="w13")
                        nc.scalar.dma_start(
                            wg[:], w13_d[i].rearrange("p (k m) -> p k m", k=KH)
                        )
                        wu = w13_pool.tile([P, KH, P], bf16, name="w13")
                        nc.scalar.dma_start(
                            wu[:],
                            w13_d[NI + i].rearrange("p (k m) -> p k m", k=KH),
                        )
                        for c in range(NCH):
                            sl = slice(c * CH, (c + 1) * CH)
                            psum_g = gu_psum.tile([P, CH], f32, name="gu")
                            psum_u = gu_psum.tile([P, CH], f32, name="gu")
                            for k in range(KH):
                                nc.tensor.matmul(
                                    psum_g[:],
                                    wg[:, k, :],
                                    x_e[:, k, sl],
                                    start=(k == 0),
                                    stop=(k == KH - 1),
                                )
                            for k in range(KH):
                                nc.tensor.matmul(
                                    psum_u[:],
                                    wu[:, k, :],
                                    x_e[:, k, sl],
                                    start=(k == 0),
                                    stop=(k == KH - 1),
                                )
                            sg = sg_pool.tile([P, CH], f32, name="sg")
                            nc.scalar.activation(sg[:], psum_g[:], Act.Silu)
                            nc.vector.tensor_tensor(
                                h_sb[:, i, sl], sg[:], psum_u[:], Alu.mult
                            )

            # ---------------- down-proj, weight, transpose, scatter -------
            with (
                tc.tile_pool(name="ydp", bufs=1) as y_pool,
                tc.tile_pool(name="w2p", bufs=3) as w2_pool,
                tc.tile_pool(name="yp", bufs=4, space="PSUM") as y_psum,
                tc.tile_pool(name="ytp", bufs=3) as yt_pool,
                tc.tile_pool(name="ytps", bufs=2, space="PSUM") as yt_psum,
            ):
                y_sb = y_pool.tile([P, HT, NSEL], bf16)
                for hh in range(HT):
                    w2t = w2_pool.tile([P, NI, P], bf16, name="w2")
                    nc.scalar.dma_start(
                        w2t[:], w2_d[hh].rearrange("p (i m) -> p i m", i=NI)
                    )
                    for c in range(NCH):
                        sl = slice(c * CH, (c + 1) * CH)
                        psum_y = y_psum.tile([P, CH], f32, name="py")
                        for i in range(NI):
                            nc.tensor.matmul(
                                psum_y[:],
                                w2t[:, i, :],
                                h_sb[:, i, sl],
                                start=(i == 0),
                                stop=(i == NI - 1),
                            )
                        nc.vector.tensor_tensor(
                            y_sb[:, hh, sl], psum_y[:], wsel_bc[:, sl], Alu.mult
                        )
                # transpose token-major and scatter into ybuf
                for j in range(JT):
                    yt = yt_pool.tile([P, H], bf16, name="yt")
                    for k in range(HT):
                        ps_y = yt_psum.tile([P, P], bf16, name="psy")
                        nc.tensor.transpose(
                            ps_y[:],
                            y_sb[:, k, j * P : (j + 1) * P],
                            ident_bf[:],
                        )
                        nc.scalar.copy(yt[:, k * P : (k + 1) * P], ps_y[:])
                    nc.gpsimd.indirect_dma_start(
                        out=ybuf[0:P, :],
                        out_offset=bass.IndirectOffsetOnAxis(
                            ap=gidx[:, j : j + 1], axis=0
                        ),
                        in_=yt[:],
                        in_offset=None,
                        bounds_check=T - 1,
                        oob_is_err=False,
                    )

            # ---------------- combine ----------------
            nc.gpsimd.collective_compute(
                "ReduceScatter",
                Alu.add,
                replica_groups=groups,
                ins=[ybuf[:].opt()],
                outs=[ybuf_rs[:].opt()],
            )
            with tc.tile_pool(name="up", bufs=2) as up_pool:
                for tt in range(TT):
                    yb = up_pool.tile([P, H], bf16, name="yb")
                    nc.sync.dma_start(yb[:], ybuf_rs[tt * P : (tt + 1) * P, :])
                    yf = up_pool.tile([P, H], f32, name="yf")
                    nc.vector.tensor_copy(yf[:], yb[:])
                    nc.sync.dma_start(out_d[tt * P : (tt + 1) * P, :], yf[:])

    return nc


def _prep_inputs(hidden_states, gate_w, w13, w2):
    """Host-side shard/layout prep (cast + transpose/slice only, no math)."""
    x = np.asarray(hidden_states, np.float32)
    gate_w = np.asarray(gate_w, np.float32)

    gw = np.ascontiguousarray(
        gate_w.reshape(E, KH, P).transpose(2, 1, 0).reshape(P, KH * E)
    )
    xg = np.ascontiguousarray(x.astype(BF16))

    w13b = np.asarray(w13).astype(BF16)
    w2b = np.asarray(w2).astype(BF16)

    in_maps = []
    for c in range(NCORES):
        xc = x[c * TC : (c + 1) * TC]
        xf = np.ascontiguousarray(
            xc.reshape(TT, P, KH, P).transpose(0, 3, 2, 1).reshape(TT, P, KH * P)
        )
        w13d = np.ascontiguousarray(
            w13b[c].reshape(MI, P, KH, P).transpose(0, 3, 2, 1).reshape(
                MI, P, KH * P
            )
        )
        w2d = np.ascontiguousarray(
            w2b[c].reshape(HT, P, NI, P).transpose(0, 3, 2, 1).reshape(
                HT, P, NI * P
            )
        )
        in_maps.append(
            {
                "xf": xf,
                "gw": gw,
                "xg": xg,
                "zb": np.zeros((1, H), BF16),
                "zf": np.zeros((1, 2), np.float32),
                "w13": w13d,
                "w2": w2d,
            }
        )
    return in_maps


def kernel(hidden_states, gate_w, w13, w2, top_k):
    import time

    assert int(top_k) == TOPK
    t0 = time.time()
    if "nc" not in _CACHE:
        nc = _build_kernel()
        _split_sync_waits(nc)
        _CACHE["nc"] = nc
    nc = _CACHE["nc"]
    t1 = time.time()
    in_maps = _prep_inputs(hidden_states, gate_w, w13, w2)
    t2 = time.time()
    res = run_bass_kernel_spmd(nc, in_maps, core_ids=list(range(NCORES)))
    t3 = time.time()
    print(
        f"[kernel] build {t1 - t0:.1f}s  prep {t2 - t1:.1f}s  run {t3 - t2:.1f}s",
        flush=True,
    )
    _CACHE["last_results"] = res

    out = np.empty((T, H), np.float32)
    for c in range(NCORES):
        out[c * TC : (c + 1) * TC] = res.results[c]["out"]
    return out
